# revision 2
# baseline (speedup 1.0000x reference)
"""ImageBEVGaussianEncoder kernel for 8 Trainium2 NeuronCores.

Sharding (per sharding_hint, adapted): data-parallel over batch and image
halves. Phase 1 runs on all 8 cores: core c processes sample c//2, image-H
half c%2 (544-row slab with receptive-field halo), running the conv encoder,
depth softmax/expected-depth, backprojection and the 9-tap Gaussian scatter
into a private per-half BEV canvas accumulator (sums + weight sums).
Phase 2 runs on 4 cores: merge the two half canvases of each sample,
normalize, and emit the (64, 256, 256) canvas.

All compute is in fp32 on-device; the host only slices/concatenates.
"""
import os
import numpy as np
import jax
import jax.numpy as jnp

# ---- constants from the module ----
OUT_C = 64
NY, NX = 256, 256
S = NY * NX
PC = (-51.2, -51.2, -5.0, 51.2, 51.2, 3.0)
VX, VY = 0.4, 0.4
DBINS, DMIN, DMAX = 16, 1.0, 60.0
SIGMA, MIN_OP, EPS = 0.8, 0.05, 1e-6
HF, WF = 64, 96           # full feature grid
H_IMG, W_IMG = 1024, 1536
SLAB_ROWS = 544           # per-core image slab height (with halo)
KEEP = 32                 # feature rows kept per core

_offs = [(dy, dx) for dy in range(-1, 2) for dx in range(-1, 2)]
OFF_DY = np.array([o[0] for o in _offs], np.int32)
OFF_DX = np.array([o[1] for o in _offs], np.int32)
KW = np.array([np.exp(-(dx * dx + dy * dy) / (2.0 * SIGMA * SIGMA)) for dy, dx in _offs],
              np.float32)

_P1 = None
_P2 = None


def _conv(x, w, stride, pad):
    return jax.lax.conv_general_dilated(
        x, w, (stride, stride), [(pad, pad), (pad, pad)],
        dimension_numbers=('NCHW', 'OIHW', 'NCHW'))


def _cbr(x, w, s, b, stride):
    y = _conv(x, w, stride, 1)
    return jax.nn.relu(y * s[None, :, None, None] + b[None, :, None, None])


def _phase1(slab, camK, Tlc, keep_off, row0,
            w1, s1, b1, w2, s2, b2, w3, s3, b3, w4, s4, b4,
            fw1, fs1, fb1, fw2, fbias2, dw, dbias, ow, obias):
    x = slab[None]                                   # (1,3,544,1536)
    x = _cbr(x, w1, s1, b1, 2)
    x = _cbr(x, w2, s2, b2, 2)
    x = _cbr(x, w3, s3, b3, 2)
    x4 = _cbr(x, w4, s4, b4, 2)                      # (1,128,34,96)
    fh = _cbr(x4, fw1, fs1, fb1, 1)
    feats = _conv(fh, fw2, 1, 0) + fbias2[None, :, None, None]   # (1,64,34,96)
    dlog = _conv(x4, dw, 1, 0) + dbias[None, :, None, None]      # (1,16,34,96)
    op = jax.nn.sigmoid(_conv(x4, ow, 1, 0) + obias[None, :, None, None])[0, 0]  # (34,96)

    # keep 32 valid feature rows for this half
    feats = jax.lax.dynamic_slice_in_dim(feats[0], keep_off, KEEP, axis=1)  # (64,32,96)
    dlog = jax.lax.dynamic_slice_in_dim(dlog[0], keep_off, KEEP, axis=1)    # (16,32,96)
    op = jax.lax.dynamic_slice_in_dim(op, keep_off, KEEP, axis=0)           # (32,96)

    dprob = jax.nn.softmax(dlog, axis=0)
    dvals = jnp.linspace(DMIN, DMAX, DBINS, dtype=jnp.float32)
    z = jnp.einsum('dhw,d->hw', dprob, dvals)        # (32,96)

    # pixel centers at global feature rows row0..row0+31
    ys = (row0 + jnp.arange(KEEP, dtype=jnp.float32) + 0.5) * (float(H_IMG) / HF)
    xs = (jnp.arange(WF, dtype=jnp.float32) + 0.5) * (float(W_IMG) / WF)
    yy, xx = jnp.meshgrid(ys, xs, indexing='ij')
    fx = jnp.maximum(camK[0, 0], EPS)
    fy = jnp.maximum(camK[1, 1], EPS)
    cx = camK[0, 2]
    cy = camK[1, 2]
    x_cam = (xx - cx) * z / fx
    y_cam = (yy - cy) * z / fy
    pts = jnp.stack([x_cam, y_cam, z, jnp.ones_like(z)], axis=-1).reshape(-1, 4)
    lidar = jnp.einsum('ij,nj->ni', Tlc, pts)[:, :3]

    xw, yw, zw = lidar[:, 0], lidar[:, 1], lidar[:, 2]
    xi = jnp.floor((xw - PC[0]) / VX).astype(jnp.int32)
    yi = jnp.floor((yw - PC[1]) / VY).astype(jnp.int32)
    inb = (xi >= 0) & (xi < NX) & (yi >= 0) & (yi < NY) & (zw >= PC[2]) & (zw < PC[5])

    opf = op.reshape(-1)
    base_w = opf * (opf >= MIN_OP) * inb

    off_dy = jnp.asarray(OFF_DY)
    off_dx = jnp.asarray(OFF_DX)
    kw = jnp.asarray(KW)
    tx = xi[None, :] + off_dx[:, None]               # (9, N)
    ty = yi[None, :] + off_dy[:, None]
    vm = (tx >= 0) & (tx < NX) & (ty >= 0) & (ty < NY)
    sw = base_w[None, :] * kw[:, None] * vm
    idx = jnp.where(vm, ty * NX + tx, 0).reshape(-1)

    featsN = feats.transpose(1, 2, 0).reshape(-1, OUT_C)   # (N, 64)
    contrib = (featsN[None] * sw[..., None]).reshape(-1, OUT_C)
    canvas = jnp.zeros((S, OUT_C), jnp.float32).at[idx].add(contrib)
    wacc = jnp.zeros((S,), jnp.float32).at[idx].add(sw.reshape(-1))
    return canvas, wacc


def _phase2(canvas2, wacc2):
    canvas = canvas2[0] + canvas2[1]                 # (S, 64)
    wacc = wacc2[0] + wacc2[1]                       # (S,)
    out = canvas / jnp.maximum(wacc, EPS)[:, None] * (wacc > 0)[:, None]
    return out.reshape(NY, NX, OUT_C).transpose(2, 0, 1)


def _get_pmapped():
    global _P1, _P2
    if _P1 is None:
        devs = jax.devices()
        wnames = 21 * (None,)
        _P1 = jax.pmap(_phase1, devices=devs[:8],
                       in_axes=(0, 0, 0, 0, 0) + wnames)
        _P2 = jax.pmap(_phase2, devices=devs[:4], in_axes=(0, 0))
    return _P1, _P2


def kernel(images, cam_K, T_lc, w1, s1, b1, w2, s2, b2, w3, s3, b3, w4, s4, b4,
           fw1, fs1, fb1, fw2, fbias2, dw, dbias, ow, obias, img_h, img_w):
    images = np.asarray(images, np.float32)
    B = images.shape[0]
    assert B == 4, "kernel hardcoded for B=4 across 8 cores"

    # host-side sharding: 544-row slabs with halo; halves at rows 0 / 480
    slabs = np.empty((8, 3, SLAB_ROWS, W_IMG), np.float32)
    camKs = np.empty((8, 3, 3), np.float32)
    Tlcs = np.empty((8, 4, 4), np.float32)
    keep_off = np.empty((8,), np.int32)
    row0 = np.empty((8,), np.float32)
    for c in range(8):
        b, h = c // 2, c % 2
        r0 = 480 * h
        slabs[c] = images[b, :, r0:r0 + SLAB_ROWS, :]
        camKs[c] = cam_K[b]
        Tlcs[c] = T_lc[b]
        keep_off[c] = 0 if h == 0 else 2
        row0[c] = 0.0 if h == 0 else 32.0

    p1, p2 = _get_pmapped()
    wargs = (w1, s1, b1, w2, s2, b2, w3, s3, b3, w4, s4, b4,
             fw1, fs1, fb1, fw2, fbias2, dw, dbias, ow, obias)
    wargs = tuple(np.asarray(a, np.float32) for a in wargs)
    canvas8, wacc8 = p1(slabs, camKs, Tlcs, keep_off, row0, *wargs)
    canvas8 = np.asarray(canvas8)
    wacc8 = np.asarray(wacc8)

    canvas_pairs = canvas8.reshape(4, 2, S, OUT_C)
    wacc_pairs = wacc8.reshape(4, 2, S)
    out4 = p2(canvas_pairs, wacc_pairs)
    return np.asarray(out4)


# revision 3
# speedup vs baseline: 2.1613x; 2.1613x over previous
"""ImageBEVGaussianEncoder kernel for 8 Trainium2 NeuronCores.

Sharding (per sharding_hint, adapted): data-parallel over batch and image
halves. Phase 1 runs on all 8 cores: core c processes sample c//2, image-H
half c%2 (544-row slab with receptive-field halo), running the conv encoder,
depth softmax/expected-depth, backprojection and the 9-tap Gaussian scatter
into a private per-half BEV canvas accumulator (sums + weight sums).
Phase 2 runs on 4 cores: merge the two half canvases of each sample,
normalize, and emit the (64, 256, 256) canvas.

All compute is in fp32 on-device; the host only slices/concatenates.
"""
import os
import numpy as np
import jax
import jax.numpy as jnp

# ---- constants from the module ----
OUT_C = 64
NY, NX = 256, 256
S = NY * NX
PC = (-51.2, -51.2, -5.0, 51.2, 51.2, 3.0)
VX, VY = 0.4, 0.4
DBINS, DMIN, DMAX = 16, 1.0, 60.0
SIGMA, MIN_OP, EPS = 0.8, 0.05, 1e-6
HF, WF = 64, 96           # full feature grid
H_IMG, W_IMG = 1024, 1536
SLAB_ROWS = 544           # per-core image slab height (with halo)
KEEP = 32                 # feature rows kept per core

_offs = [(dy, dx) for dy in range(-1, 2) for dx in range(-1, 2)]
OFF_DY = np.array([o[0] for o in _offs], np.int32)
OFF_DX = np.array([o[1] for o in _offs], np.int32)
KW = np.array([np.exp(-(dx * dx + dy * dy) / (2.0 * SIGMA * SIGMA)) for dy, dx in _offs],
              np.float32)

_P1 = None
_P2 = None


def _conv(x, w, stride, pad):
    return jax.lax.conv_general_dilated(
        x, w, (stride, stride), [(pad, pad), (pad, pad)],
        dimension_numbers=('NCHW', 'OIHW', 'NCHW'))


def _cbr(x, w, s, b, stride):
    y = _conv(x, w, stride, 1)
    return jax.nn.relu(y * s[None, :, None, None] + b[None, :, None, None])


def _phase1(slab, camK, Tlc, keep_off, row0,
            w1, s1, b1, w2, s2, b2, w3, s3, b3, w4, s4, b4,
            fw1, fs1, fb1, fw2, fbias2, dw, dbias, ow, obias):
    x = slab[None]                                   # (1,3,544,1536)
    x = _cbr(x, w1, s1, b1, 2)
    x = _cbr(x, w2, s2, b2, 2)
    x = _cbr(x, w3, s3, b3, 2)
    x4 = _cbr(x, w4, s4, b4, 2)                      # (1,128,34,96)
    fh = _cbr(x4, fw1, fs1, fb1, 1)
    feats = _conv(fh, fw2, 1, 0) + fbias2[None, :, None, None]   # (1,64,34,96)
    dlog = _conv(x4, dw, 1, 0) + dbias[None, :, None, None]      # (1,16,34,96)
    op = jax.nn.sigmoid(_conv(x4, ow, 1, 0) + obias[None, :, None, None])[0, 0]  # (34,96)

    # keep 32 valid feature rows for this half
    feats = jax.lax.dynamic_slice_in_dim(feats[0], keep_off, KEEP, axis=1)  # (64,32,96)
    dlog = jax.lax.dynamic_slice_in_dim(dlog[0], keep_off, KEEP, axis=1)    # (16,32,96)
    op = jax.lax.dynamic_slice_in_dim(op, keep_off, KEEP, axis=0)           # (32,96)

    dprob = jax.nn.softmax(dlog, axis=0)
    dvals = jnp.linspace(DMIN, DMAX, DBINS, dtype=jnp.float32)
    z = jnp.einsum('dhw,d->hw', dprob, dvals)        # (32,96)

    # pixel centers at global feature rows row0..row0+31
    ys = (row0 + jnp.arange(KEEP, dtype=jnp.float32) + 0.5) * (float(H_IMG) / HF)
    xs = (jnp.arange(WF, dtype=jnp.float32) + 0.5) * (float(W_IMG) / WF)
    yy, xx = jnp.meshgrid(ys, xs, indexing='ij')
    fx = jnp.maximum(camK[0, 0], EPS)
    fy = jnp.maximum(camK[1, 1], EPS)
    cx = camK[0, 2]
    cy = camK[1, 2]
    x_cam = (xx - cx) * z / fx
    y_cam = (yy - cy) * z / fy
    pts = jnp.stack([x_cam, y_cam, z, jnp.ones_like(z)], axis=-1).reshape(-1, 4)
    lidar = jnp.einsum('ij,nj->ni', Tlc, pts)[:, :3]

    xw, yw, zw = lidar[:, 0], lidar[:, 1], lidar[:, 2]
    xi = jnp.floor((xw - PC[0]) / VX).astype(jnp.int32)
    yi = jnp.floor((yw - PC[1]) / VY).astype(jnp.int32)
    inb = (xi >= 0) & (xi < NX) & (yi >= 0) & (yi < NY) & (zw >= PC[2]) & (zw < PC[5])

    opf = op.reshape(-1)
    base_w = opf * (opf >= MIN_OP) * inb

    off_dy = jnp.asarray(OFF_DY)
    off_dx = jnp.asarray(OFF_DX)
    kw = jnp.asarray(KW)
    tx = xi[None, :] + off_dx[:, None]               # (9, N)
    ty = yi[None, :] + off_dy[:, None]
    vm = (tx >= 0) & (tx < NX) & (ty >= 0) & (ty < NY)
    sw = base_w[None, :] * kw[:, None] * vm
    idx = jnp.where(vm, ty * NX + tx, 0).reshape(-1)

    featsN = feats.transpose(1, 2, 0).reshape(-1, OUT_C)   # (N, 64)
    contrib = (featsN[None] * sw[..., None]).reshape(-1, OUT_C)
    canvas = jnp.zeros((S, OUT_C), jnp.float32).at[idx].add(contrib)
    wacc = jnp.zeros((S,), jnp.float32).at[idx].add(sw.reshape(-1))

    # merge the two half-image canvases of this sample on-device, then each
    # core normalizes and emits only its own half of the BEV rows.
    groups = [[0, 1], [2, 3], [4, 5], [6, 7]]
    canvas = jax.lax.psum(canvas, 'cores', axis_index_groups=groups)
    wacc = jax.lax.psum(wacc, 'cores', axis_index_groups=groups)
    half_rows = S // 2
    row_start = keep_off * (half_rows // 2)          # keep_off: 0 -> 0, 2 -> 32768
    chalf = jax.lax.dynamic_slice_in_dim(canvas, row_start, half_rows, axis=0)
    whalf = jax.lax.dynamic_slice_in_dim(wacc, row_start, half_rows, axis=0)
    out = chalf / jnp.maximum(whalf, EPS)[:, None] * (whalf > 0)[:, None]
    return out.reshape(NY // 2, NX, OUT_C).transpose(2, 0, 1)    # (64,128,256)


def _get_pmapped():
    global _P1
    if _P1 is None:
        devs = jax.devices()
        wnames = 21 * (None,)
        _P1 = jax.pmap(_phase1, axis_name='cores', devices=devs[:8],
                       in_axes=(0, 0, 0, 0, 0) + wnames)
    return _P1


def kernel(images, cam_K, T_lc, w1, s1, b1, w2, s2, b2, w3, s3, b3, w4, s4, b4,
           fw1, fs1, fb1, fw2, fbias2, dw, dbias, ow, obias, img_h, img_w):
    images = np.asarray(images, np.float32)
    B = images.shape[0]
    assert B == 4, "kernel hardcoded for B=4 across 8 cores"

    # host-side sharding: 544-row slabs with halo; halves at rows 0 / 480
    slabs = np.empty((8, 3, SLAB_ROWS, W_IMG), np.float32)
    camKs = np.empty((8, 3, 3), np.float32)
    Tlcs = np.empty((8, 4, 4), np.float32)
    keep_off = np.empty((8,), np.int32)
    row0 = np.empty((8,), np.float32)
    for c in range(8):
        b, h = c // 2, c % 2
        r0 = 480 * h
        slabs[c] = images[b, :, r0:r0 + SLAB_ROWS, :]
        camKs[c] = cam_K[b]
        Tlcs[c] = T_lc[b]
        keep_off[c] = 0 if h == 0 else 2
        row0[c] = 0.0 if h == 0 else 32.0

    p1 = _get_pmapped()
    wargs = (w1, s1, b1, w2, s2, b2, w3, s3, b3, w4, s4, b4,
             fw1, fs1, fb1, fw2, fbias2, dw, dbias, ow, obias)
    wargs = tuple(np.asarray(a, np.float32) for a in wargs)
    out8 = np.asarray(p1(slabs, camKs, Tlcs, keep_off, row0, *wargs))
    # out8: (8, 64, 128, 256); core 2b has sample b BEV rows 0..127, 2b+1 rows 128..255
    return np.concatenate(
        [np.concatenate([out8[2 * b], out8[2 * b + 1]], axis=1)[None] for b in range(B)],
        axis=0)
